# revision 1
# baseline (speedup 1.0000x reference)
"""Trainium2 Bass kernel for MergedQKVParallelLinearWithLoRA.

Computes out = x @ W_qkv^T + b_qkv + per-token-LoRA, where each token t uses
adapter l_t = lora_indices[t]:
    shrink_s = x @ A_s[l_t]^T            (R=16 per slice s in {q,k,v})
    out[:, slice_s] += shrink_s @ B_s[l_t]^T

Strategy (8 NeuronCores, token-parallel):
  - Each core handles 1024 tokens, all 6144 output columns.
  - Host pre-transposes: xT [H, Tc] per core (bf16), wT [H, OUT] quantized to
    int8 with one global scale s_w, aT [H, 3*L*R] int8 (scale s_a),
    bT [L*R, OUT] bf16 (per-slice packed), plus a one-hot adapter mask.
    The mask rows repeat across q/k/v so only [2*128, Tc] is uploaded, and
    its nonzero value is s_a/s_w. s_w itself is folded into x on the host
    (bf16 is scale-free), so PSUM accumulates the final unscaled output
    directly and the epilogue is a plain DVE add of the bf16 bias tile.
  - int8 tiles are cast to bf16 on DVE before the PE (PE takes no int8).
    All matmuls are bf16 (1 cycle/row), PSUM accumulates fp32, out is
    stored bf16 and upcast on host.
"""

import numpy as np

T = 8192
H = 4096
OUT_Q = 4096
OUT_KV = 1024
OUT = OUT_Q + 2 * OUT_KV  # 6144
L = 16
R = 16
LR3 = 3 * L * R  # 768
NCORES = 8
TC = T // NCORES  # 1024

_cache = {}


def _build(h, out_q, out_kv, tc_tokens, reps=1, timing_inputs=False, skip_lora=False, skip_main=False):
    """Build the per-core Bass program. All cores run the same NEFF (SPMD).

    reps > 1 wraps the whole body in a device-side For_i loop — used by the
    test harness to measure per-iteration HW time via wall-clock deltas.
    timing_inputs=True declares inputs as Internal DRAM (uninitialized, no
    host transfer) so wall-clock deltas are dominated by device exec time.
    """
    import concourse.bass as bass  # noqa: F401
    import concourse.mybir as mybir
    import concourse.tile as tile
    from concourse import bacc

    f32 = mybir.dt.float32
    bf16 = mybir.dt.bfloat16
    i8 = mybir.dt.int8

    out_total = out_q + 2 * out_kv
    NH = h // 128          # contraction tiles
    NT = tc_tokens // 128  # token tiles (output partition dim)
    NOB = out_total // 512  # output column blocks
    NQB = out_q // 512      # q blocks
    NKB = out_kv // 512     # k blocks
    NC512 = tc_tokens // 512  # 512-token chunks for shrink
    NJ = LR3 // 128        # 6 lr tiles

    assert out_q % 512 == 0 and out_kv % 512 == 0 and tc_tokens % 512 == 0

    nc = bacc.Bacc(None, target_bir_lowering=False)

    in_kw = {} if timing_inputs else {"kind": "ExternalInput"}
    xT = nc.dram_tensor("xT", [h, tc_tokens], bf16, **in_kw)
    w8 = nc.dram_tensor("w8", [h, out_total], i8, **in_kw)
    a8 = nc.dram_tensor("a8", [h, LR3], i8, **in_kw)
    bT = nc.dram_tensor("bT", [2 * 128, out_total], bf16, **in_kw)
    # mask rows repeat 3x across q/k/v slices -> only 2 tiles uploaded;
    # nonzero value is s_a/s_w (folds both int8 scales)
    maskT = nc.dram_tensor("maskT", [2 * 128, tc_tokens], bf16, **in_kw)
    biasb = nc.dram_tensor("biasb", [128, out_total], bf16, **in_kw)
    # out is stored bf16 (host upcasts to fp32) — halves the store traffic
    if timing_inputs:
        # keep the big result internal; expose only a tiny sink so per-call
        # host<->device transfer stays negligible for wall-delta timing
        out = nc.dram_tensor("out", [tc_tokens, out_total], bf16)
        sink = nc.dram_tensor("sink", [128, 512], bf16, kind="ExternalOutput")
    else:
        out = nc.dram_tensor(
            "out", [tc_tokens, out_total], bf16, kind="ExternalOutput"
        )
        sink = None

    with tile.TileContext(nc) as tc:
        from contextlib import ExitStack

        with ExitStack() as ctx:
            xp = ctx.enter_context(tc.tile_pool(name="xp", bufs=1))
            sp = ctx.enter_context(tc.tile_pool(name="sp", bufs=1))
            pp = ctx.enter_context(tc.tile_pool(name="pp", bufs=8, space="PSUM"))
            atp = ctx.enter_context(tc.tile_pool(name="atp", bufs=1))
            mp = ctx.enter_context(tc.tile_pool(name="mp", bufs=2))
            wp = ctx.enter_context(tc.tile_pool(name="wp", bufs=8))
            wbp = ctx.enter_context(tc.tile_pool(name="wbp", bufs=4))
            btp = ctx.enter_context(tc.tile_pool(name="btp", bufs=3))
            bp2 = ctx.enter_context(tc.tile_pool(name="bp2", bufs=2))
            op = ctx.enter_context(tc.tile_pool(name="op", bufs=16))

            loop_ctx = tc.For_i(0, reps, 1) if reps > 1 else None
            if loop_ctx is not None:
                loop_ctx.__enter__()

            # Resident x^T: [128, NH, Tc] bf16 (partition = h % 128)
            xT_sb = xp.tile([128, NH, tc_tokens], bf16, name="xT_sb", tag="xT_sb")
            for a in range(NH):
                nc.sync.dma_start(
                    xT_sb[:, a, :], xT[a * 128:(a + 1) * 128, :]
                )
            # Resident a^T int8 + bf16 cast: loaded once (3.1 MB DMA)
            at8_sb = atp.tile([128, NH, LR3], i8, name="at8_sb", tag="at8_sb")
            at_sb = atp.tile([128, NH, LR3], bf16, name="at_sb", tag="at_sb")
            for a in range(NH if not skip_lora else 0):
                nc.sync.dma_start(
                    at8_sb[:, a, :], a8[a * 128:(a + 1) * 128, :]
                )
                nc.vector.tensor_copy(at_sb[:, a, :], at8_sb[:, a, :])
            # Resident masked shrink^T: [128, NJ, Tc] bf16
            shrT = sp.tile([128, NJ, tc_tokens], bf16, name="shrT", tag="shrT")

            # ---- Phase 1: LoRA shrink (dense over adapters) + mask ----
            for th in range(NC512 if not skip_lora else 0):
                tsl = slice(th * 512, (th + 1) * 512)
                ps = [
                    pp.tile([128, 512], f32, name=f"shps_{th}_{j}", tag="ps")
                    for j in range(NJ)
                ]
                for hh in range(NH):
                    for j in range(NJ):
                        nc.tensor.matmul(
                            ps[j][:],
                            at_sb[:, hh, j * 128:(j + 1) * 128],
                            xT_sb[:, hh, tsl],
                            start=(hh == 0),
                            stop=(hh == NH - 1),
                        )
                ms = []
                for q in range(2):
                    m = mp.tile([128, 512], bf16, name=f"m_{th}_{q}", tag="m")
                    nc.sync.dma_start(m, maskT[q * 128:(q + 1) * 128, tsl])
                    ms.append(m)
                for j in range(NJ):
                    nc.vector.tensor_mul(shrT[:, j, tsl], ps[j][:], ms[j % 2][:])

            # ---- Phase 2: base GEMM + LoRA expand + bias ----
            # Out-stores are deferred by one ob and interleaved into the next
            # ob's weight loop: by then their DVE-mul deps are long done, so
            # they never head-block the ACT queue's w8 prefetch stream.
            pending_stores = []
            for ob in range(NOB if not skip_main else 0):
                osl = slice(ob * 512, (ob + 1) * 512)
                # which slice (q/k/v) this 512-col block belongs to
                if ob < NQB:
                    jbase = 0
                elif ob < NQB + NKB:
                    jbase = 2
                else:
                    jbase = 4
                ps = [
                    pp.tile([128, 512], f32, name=f"mps_{ob}_{t}", tag="ps")
                    for t in range(NT)
                ]
                for hh in range(NH):
                    w8t = wp.tile([128, 512], i8, name=f"w8_{ob}_{hh}", tag="w8")
                    # alternate the two HWDGE queues so weight loads use both
                    eng = nc.sync if hh % 2 == 0 else nc.scalar
                    eng.dma_start(w8t, w8[hh * 128:(hh + 1) * 128, osl])
                    w = wbp.tile([128, 512], bf16, name=f"w_{ob}_{hh}", tag="w")
                    nc.vector.tensor_copy(w[:], w8t[:])
                    if pending_stores and hh % 2 == 1:
                        po, pt, posl = pending_stores.pop(0)
                        nc.scalar.dma_start(
                            out[pt * 128:(pt + 1) * 128, posl], po[:]
                        )
                    for t in range(NT):
                        nc.tensor.matmul(
                            ps[t][:],
                            xT_sb[:, hh, t * 128:(t + 1) * 128],
                            w[:],
                            start=(hh == 0),
                            stop=(skip_lora and hh == NH - 1),
                        )
                for jj in range(2 if not skip_lora else 0):
                    bt = btp.tile([128, 512], bf16, name=f"bt_{ob}_{jj}", tag="bt")
                    nc.sync.dma_start(
                        bt, bT[jj * 128:(jj + 1) * 128, osl]
                    )
                    for t in range(NT):
                        nc.tensor.matmul(
                            ps[t][:],
                            shrT[:, jbase + jj, t * 128:(t + 1) * 128],
                            bt[:],
                            start=False,
                            stop=(jj == 1),
                        )
                bias_t = bp2.tile([128, 512], bf16, name=f"bias_{ob}", tag="bias")
                nc.sync.dma_start(bias_t, biasb[:, osl])
                for t in range(NT):
                    o = op.tile([128, 512], bf16, name=f"o_{ob}_{t}", tag="o")
                    nc.vector.tensor_add(o[:], ps[t][:], bias_t[:])
                    pending_stores.append((o, t, osl))

            # flush the last ob's stores
            for po, pt, posl in pending_stores:
                nc.scalar.dma_start(out[pt * 128:(pt + 1) * 128, posl], po[:])
            pending_stores = []

            if loop_ctx is not None:
                loop_ctx.__exit__(None, None, None)

            if sink is not None:
                nc.scalar.dma_start(sink[:], out[0:128, 0:512])

    nc.compile()
    return nc


def _get_nc(h=H, out_q=OUT_Q, out_kv=OUT_KV, tc_tokens=TC, reps=1,
            timing_inputs=False, skip_lora=False, skip_main=False):
    key = (h, out_q, out_kv, tc_tokens, reps, timing_inputs, skip_lora, skip_main)
    if key not in _cache:
        _cache[key] = _build(
            h, out_q, out_kv, tc_tokens, reps=reps, timing_inputs=timing_inputs,
            skip_lora=skip_lora, skip_main=skip_main,
        )
    return _cache[key]


def _host_prep(x, w_qkv, b_qkv, a_q, a_k, a_v, b_q, b_k, b_v, lora_indices,
               n_cores=NCORES):
    """Build per-core input maps (host-side transposes/packing)."""
    import ml_dtypes

    f = np.float32
    bf = ml_dtypes.bfloat16
    x = np.ascontiguousarray(np.asarray(x, f))
    t_total, h = x.shape
    tc_tokens = t_total // n_cores
    out_q = np.asarray(b_q).shape[1]
    out_kv = np.asarray(b_k).shape[1]
    out_total = out_q + 2 * out_kv

    def _qscale(arr):
        # clip at 4 sigma: the rare clipped tail costs less error than the
        # coarser quantization step an absmax scale would force
        amax = float(np.abs(arr).max())
        clip = min(amax, 4.0 * float(arr.std()))
        return (clip / 127.0) if clip > 0 else 1.0

    w_f = np.asarray(w_qkv, f)
    s_w = _qscale(w_f)
    w8 = np.ascontiguousarray(
        np.clip(np.round(w_f.T / s_w), -127, 127).astype(np.int8)
    )  # [H, OUT]

    l, r = np.asarray(a_q).shape[:2]
    a_f = np.concatenate(
        [np.asarray(a, f).reshape(l * r, h) for a in (a_q, a_k, a_v)], axis=0
    )  # [3LR, H]
    s_a = _qscale(a_f)
    a8 = np.ascontiguousarray(
        np.clip(np.round(a_f.T / s_a), -127, 127).astype(np.int8)
    )  # [H, 3LR]

    bT = np.ascontiguousarray(
        np.concatenate(
            [
                np.asarray(b, f).transpose(0, 2, 1).reshape(l * r, -1)
                for b in (b_q, b_k, b_v)
            ],
            axis=1,
        ).astype(bf)
    )  # [L*R, OUT]
    biasb = np.ascontiguousarray(
        np.broadcast_to(np.asarray(b_qkv, f).astype(bf), (128, out_total))
    )

    li = np.asarray(lora_indices).astype(np.int64)
    # mask nonzero value folds both int8 scales: s_a/s_w
    oh = (li[:, None] == np.arange(l)[None, :]).astype(f) * np.float32(s_a / s_w)
    mask_exp = np.repeat(oh, r, axis=1).astype(bf)               # [T, L*R]
    maskT_full = np.ascontiguousarray(mask_exp.T)                # [2*128, T]

    # s_w is folded into x (bf16 is scale-free): PSUM then holds the final
    # unscaled output directly and no per-element rescale is needed
    xT_bf = np.ascontiguousarray((x.T * np.float32(s_w)).astype(bf))  # [H, T]

    in_maps = []
    for c in range(n_cores):
        tsl = slice(c * tc_tokens, (c + 1) * tc_tokens)
        in_maps.append(
            {
                "xT": np.ascontiguousarray(xT_bf[:, tsl]),
                "w8": w8,
                "a8": a8,
                "bT": bT,
                "maskT": np.ascontiguousarray(maskT_full[:, tsl]),
                "biasb": biasb,
            }
        )
    return in_maps


def kernel(x, w_qkv, b_qkv, a_q, a_k, a_v, b_q, b_k, b_v, lora_indices):
    from concourse.bass_utils import run_bass_kernel_spmd

    in_maps = _host_prep(
        x, w_qkv, b_qkv, a_q, a_k, a_v, b_q, b_k, b_v, lora_indices
    )
    nc = _get_nc()
    core_ids = list(range(NCORES))
    res = run_bass_kernel_spmd(nc, in_maps, core_ids)
    return np.concatenate(
        [np.asarray(res.results[c]["out"], dtype=np.float32) for c in core_ids],
        axis=0,
    )



# revision 2
# speedup vs baseline: 1.3504x; 1.3504x over previous
"""Trainium2 Bass kernel for MergedQKVParallelLinearWithLoRA.

Computes out = x @ W_qkv^T + b_qkv + per-token-LoRA, where each token t uses
adapter l_t = lora_indices[t]:
    shrink_s = x @ A_s[l_t]^T            (R=16 per slice s in {q,k,v})
    out[:, slice_s] += shrink_s @ B_s[l_t]^T

Strategy (8 NeuronCores, token-parallel), v2 "column-stationary" layout:
  - Each core handles 1024 tokens, all 6144 output columns.
  - Main GEMM runs transposed: stationary = W tile [128h, 128out] (int8
    streamed from HBM, cast to bf16 on DVE), moving = resident x^T bf16.
    PSUM holds [128 out-cols, 512 tok] tiles -> output-channel partitions.
    Epilogue runs on the Scalar engine (ACT): psum + per-partition bias ->
    bf16 staging -> DMA store of out^T [OUT, Tc]; host transposes back.
    This keeps DVE (casts) / ACT (epilogue+stores) / PE (matmuls) on
    disjoint critical paths so PSUM banks recycle without stalling the PE.
  - LoRA shrink and expand use fp8e4m3 DoubleRow matmuls (2 contraction
    rows/cycle, ~1.9x measured): x8 = fp8(x/s_x), a8 = fp8(aT/s_a) pairs
    over h; masked shrink is written by DVE directly as fp8 at scale
    alpha (shr8 = shrink*alpha), expand uses b8 = fp8(bT/alpha) so the
    psum contribution lands at true scale. The adapter one-hot mask value
    folds s_x*s_a*alpha.
  - W int8 uses one global 4-sigma-clipped scale s_w folded into the bf16
    x^T upload, so PSUM accumulates the final unscaled output directly.
"""

import numpy as np

T = 8192
H = 4096
OUT_Q = 4096
OUT_KV = 1024
OUT = OUT_Q + 2 * OUT_KV  # 6144
L = 16
R = 16
LR3 = 3 * L * R  # 768
NCORES = 8
TC = T // NCORES  # 1024

NH = H // 128        # 32 h tiles
NH2 = NH // 2        # 16 h pair-tiles (DoubleRow)
NOB = OUT // 512     # 12 output 512-col blocks
NJ = LR3 // 128      # 6 lr tiles

_cache = {}


def _build(reps=1, timing_inputs=False, skip_lora=False, skip_main=False):
    """Build the per-core Bass program. All cores run the same NEFF (SPMD).

    reps > 1 wraps the whole body in a device-side For_i loop — used by the
    test harness to measure per-iteration HW time via wall-clock deltas.
    timing_inputs=True declares inputs as Internal DRAM (uninitialized, no
    host transfer) so wall-clock deltas are dominated by device exec time.
    """
    import concourse.bass as bass  # noqa: F401
    import concourse.mybir as mybir
    import concourse.tile as tile
    from concourse import bacc

    f32 = mybir.dt.float32
    bf16 = mybir.dt.bfloat16
    i8 = mybir.dt.int8
    fp8 = mybir.dt.float8e4
    DR = mybir.MatmulPerfMode.DoubleRow

    nc = bacc.Bacc(None, target_bir_lowering=False)

    in_kw = {} if timing_inputs else {"kind": "ExternalInput"}
    xT = nc.dram_tensor("xT", [H, TC], bf16, **in_kw)
    w8 = nc.dram_tensor("w8", [H, OUT], i8, **in_kw)
    x8 = nc.dram_tensor("x8", [H, TC], fp8, **in_kw)
    a8 = nc.dram_tensor("a8", [H, LR3], fp8, **in_kw)
    b8 = nc.dram_tensor("b8", [2 * 128, OUT], fp8, **in_kw)
    maskT = nc.dram_tensor("maskT", [2 * 128, TC], bf16, **in_kw)
    biasv = nc.dram_tensor("biasv", [128, NOB * 4], f32, **in_kw)
    # out is stored transposed [OUT, Tc] bf16 (host transposes + upcasts)
    if timing_inputs:
        out = nc.dram_tensor("out", [OUT, TC], bf16)
        sink = nc.dram_tensor("sink", [128, 512], bf16, kind="ExternalOutput")
    else:
        out = nc.dram_tensor("out", [OUT, TC], bf16, kind="ExternalOutput")
        sink = None

    with tile.TileContext(nc) as tc:
        from contextlib import ExitStack

        with ExitStack() as ctx:
            xp = ctx.enter_context(tc.tile_pool(name="xp", bufs=1))
            x8pool = ctx.enter_context(tc.tile_pool(name="x8p", bufs=1))
            apool = ctx.enter_context(tc.tile_pool(name="ap", bufs=1))
            bpool = ctx.enter_context(tc.tile_pool(name="bp", bufs=1))
            mpool = ctx.enter_context(tc.tile_pool(name="mp", bufs=1))
            spool = ctx.enter_context(tc.tile_pool(name="sp", bufs=1))
            pp = ctx.enter_context(tc.tile_pool(name="pp", bufs=8, space="PSUM"))
            wp = ctx.enter_context(tc.tile_pool(name="wp", bufs=8))
            wbp = ctx.enter_context(tc.tile_pool(name="wbp", bufs=8))
            op = ctx.enter_context(tc.tile_pool(name="op", bufs=12))

            loop_ctx = tc.For_i(0, reps, 1) if reps > 1 else None
            if loop_ctx is not None:
                loop_ctx.__enter__()

            # ---- resident loads (scalar/ACT HWDGE ring, dependency order:
            # phase-1 inputs first so the PE can start ~immediately) ----
            maskT_sb = mpool.tile([128, 2, TC], bf16, name="maskT_sb", tag="mk")
            biasv_sb = mpool.tile([128, NOB * 4], f32, name="biasv_sb", tag="bv")
            x8p = x8pool.tile([128, NH2, 2, TC], fp8, name="x8p", tag="x8p")
            at8 = apool.tile([128, NH2, 2, LR3], fp8, name="at8", tag="at8")
            bt8 = bpool.tile([128, 2, OUT], fp8, name="bt8", tag="bt8")
            xT_sb = xp.tile([128, NH, TC], bf16, name="xT_sb", tag="xT_sb")
            shrT8 = spool.tile([128, 3, 2, TC], fp8, name="shrT8", tag="shrT8")

            if not skip_lora:
                for i in range(2):
                    nc.scalar.dma_start(
                        maskT_sb[:, i, :], maskT[i * 128:(i + 1) * 128, :]
                    )
                for k in range(NH2):
                    for i in range(2):
                        r0 = (2 * k + i) * 128
                        nc.scalar.dma_start(
                            x8p[:, k, i, :], x8[r0:r0 + 128, :]
                        )
                        nc.scalar.dma_start(
                            at8[:, k, i, :], a8[r0:r0 + 128, :]
                        )
            if not skip_main:
                nc.scalar.dma_start(biasv_sb[:], biasv[:, :])
                for a in range(NH):
                    nc.scalar.dma_start(
                        xT_sb[:, a, :], xT[a * 128:(a + 1) * 128, :]
                    )
            if not skip_lora:
                for i in range(2):
                    nc.scalar.dma_start(
                        bt8[:, i, :], b8[i * 128:(i + 1) * 128, :]
                    )

            # ---- Phase 1: LoRA shrink (DoubleRow fp8, dense over adapters),
            # masked + written as fp8 at scale alpha ----
            for th in range(2 if not skip_lora else 0):
                tsl = slice(th * 512, (th + 1) * 512)
                ps6 = [
                    pp.tile([128, 512], f32, name=f"shps_{th}_{j}", tag="ps")
                    for j in range(NJ)
                ]
                for k in range(NH2):
                    for j in range(NJ):
                        nc.tensor.matmul(
                            ps6[j][:],
                            at8[:, k, :, j * 128:(j + 1) * 128],
                            x8p[:, k, :, tsl],
                            start=(k == 0),
                            stop=(k == NH2 - 1),
                            perf_mode=DR,
                        )
                for j in range(NJ):
                    nc.vector.tensor_mul(
                        shrT8[:, j // 2, j % 2, tsl],
                        ps6[j][:],
                        maskT_sb[:, j % 2, tsl],
                    )

            # ---- Phase 2: base GEMM (W-stationary) + LoRA expand + bias ----
            for ob in range(NOB if not skip_main else 0):
                osl = slice(ob * 512, (ob + 1) * 512)
                sidx = 0 if ob < 8 else (1 if ob < 10 else 2)
                ps = [
                    [
                        pp.tile([128, 512], f32, name=f"mps_{ob}_{sub}_{th}",
                                tag="ps")
                        for th in range(2)
                    ]
                    for sub in range(4)
                ]
                for hh in range(NH):
                    w8t = wp.tile([128, 512], i8, name=f"w8_{ob}_{hh}", tag="w8")
                    nc.sync.dma_start(w8t, w8[hh * 128:(hh + 1) * 128, osl])
                    wb = wbp.tile([128, 512], bf16, name=f"w_{ob}_{hh}", tag="w")
                    nc.vector.tensor_copy(wb[:], w8t[:])
                    for sub in range(4):
                        for th in range(2):
                            nc.tensor.matmul(
                                ps[sub][th][:],
                                wb[:, sub * 128:(sub + 1) * 128],
                                xT_sb[:, hh, th * 512:(th + 1) * 512],
                                start=(hh == 0),
                                stop=(skip_lora and hh == NH - 1),
                            )
                for sub in range(4):
                    c0 = ob * 512 + sub * 128
                    cidx = ob * 4 + sub
                    for th in range(2):
                        if not skip_lora:
                            nc.tensor.matmul(
                                ps[sub][th][:],
                                bt8[:, :, c0:c0 + 128],
                                shrT8[:, sidx, :, th * 512:(th + 1) * 512],
                                start=False,
                                stop=True,
                                perf_mode=DR,
                            )
                        ot = op.tile([128, 512], bf16,
                                     name=f"o_{ob}_{sub}_{th}", tag="o")
                        nc.scalar.add(
                            ot[:], ps[sub][th][:],
                            add=biasv_sb[:, cidx:cidx + 1],
                        )
                        nc.scalar.dma_start(
                            out[c0:c0 + 128, th * 512:(th + 1) * 512], ot[:]
                        )

            if loop_ctx is not None:
                loop_ctx.__exit__(None, None, None)

            if sink is not None:
                nc.scalar.dma_start(sink[:], out[0:128, 0:512])

    nc.compile()
    return nc


def _get_nc(reps=1, timing_inputs=False, skip_lora=False, skip_main=False):
    key = (reps, timing_inputs, skip_lora, skip_main)
    if key not in _cache:
        _cache[key] = _build(
            reps=reps, timing_inputs=timing_inputs,
            skip_lora=skip_lora, skip_main=skip_main,
        )
    return _cache[key]


def _host_prep(x, w_qkv, b_qkv, a_q, a_k, a_v, b_q, b_k, b_v, lora_indices,
               n_cores=NCORES):
    """Build per-core input maps (host-side transposes/packing/quantization)."""
    import ml_dtypes

    f = np.float32
    bf = ml_dtypes.bfloat16
    e4 = ml_dtypes.float8_e4m3  # TRN FP8_EXP4: max +-240

    x = np.ascontiguousarray(np.asarray(x, f))
    t_total, h = x.shape
    tc_tokens = t_total // n_cores

    def _to8(arr):
        return np.clip(np.asarray(arr, f), -240.0, 240.0).astype(e4)

    # main GEMM: int8 W with 4-sigma-clipped global scale, folded into x^T
    w_f = np.asarray(w_qkv, f)
    amax = float(np.abs(w_f).max())
    clip = min(amax, 4.0 * float(w_f.std()))
    s_w = (clip / 127.0) if clip > 0 else 1.0
    w8 = np.ascontiguousarray(
        np.clip(np.round(w_f.T / s_w), -127, 127).astype(np.int8)
    )  # [H, OUT]
    xT_bf = np.ascontiguousarray((x.T * np.float32(s_w)).astype(bf))  # [H, T]

    # LoRA path, all fp8e4m3
    l, r = np.asarray(a_q).shape[:2]
    a_f = np.concatenate(
        [np.asarray(a, f).reshape(l * r, h) for a in (a_q, a_k, a_v)], axis=0
    )  # [3LR, H]
    s_x8 = float(np.abs(x).max()) / 200.0
    s_a8 = float(np.abs(a_f).max()) / 200.0
    alpha = 0.1
    x8 = np.ascontiguousarray(_to8(x.T / s_x8))          # [H, T]
    a8 = np.ascontiguousarray(_to8(a_f.T / s_a8))        # [H, 3LR]
    bT = np.concatenate(
        [
            np.asarray(b, f).transpose(0, 2, 1).reshape(l * r, -1)
            for b in (b_q, b_k, b_v)
        ],
        axis=1,
    )  # [L*R, OUT]
    b8 = np.ascontiguousarray(_to8(bT / alpha))          # [256, OUT]

    li = np.asarray(lora_indices).astype(np.int64)
    m_val = np.float32(s_x8 * s_a8 * alpha)
    oh = (li[:, None] == np.arange(l)[None, :]).astype(f) * m_val
    mask_exp = np.repeat(oh, r, axis=1).astype(bf)       # [T, L*R]
    maskT_full = np.ascontiguousarray(mask_exp.T)        # [256, T]

    out_total = bT.shape[1]
    biasv = np.ascontiguousarray(
        np.asarray(b_qkv, f).reshape(out_total // 128, 128).T
    )  # [128, 48]

    in_maps = []
    for c in range(n_cores):
        tsl = slice(c * tc_tokens, (c + 1) * tc_tokens)
        in_maps.append(
            {
                "xT": np.ascontiguousarray(xT_bf[:, tsl]),
                "w8": w8,
                "x8": np.ascontiguousarray(x8[:, tsl]),
                "a8": a8,
                "b8": b8,
                "maskT": np.ascontiguousarray(maskT_full[:, tsl]),
                "biasv": biasv,
            }
        )
    return in_maps


def kernel(x, w_qkv, b_qkv, a_q, a_k, a_v, b_q, b_k, b_v, lora_indices):
    from concourse.bass_utils import run_bass_kernel_spmd

    in_maps = _host_prep(
        x, w_qkv, b_qkv, a_q, a_k, a_v, b_q, b_k, b_v, lora_indices
    )
    nc = _get_nc()
    core_ids = list(range(NCORES))
    res = run_bass_kernel_spmd(nc, in_maps, core_ids)
    return np.concatenate(
        [
            np.asarray(res.results[c]["out"], dtype=np.float32).T
            for c in core_ids
        ],
        axis=0,
    )
